# revision 9
# baseline (speedup 1.0000x reference)
"""CRPS loss kernel for Trainium2 (8 NeuronCores, pure data parallel).

Math: for each row i with logits x and label t,
    probs = softmax(x);  F = cumsum(probs);  mask_j = 1[j >= t]
    loss_i = sum_j (F_j - mask_j)^2
    output = mean over all (i, j) = sum_i loss_i / (B*C)

Device formulation (per row, one DVE prefix-scan):
    e = exp(x)                  ACT, fused accum_out -> s = sum(e)
    imp_j = s * (iota_j == t)   one 4x tensor_scalar op
    st_j = (e_j + st_{j-1}) - imp_j    = cumsum(e)_j - s*mask_j   (DVE scan)
    A = sum_j st_j^2            ACT Square accum (11/16 tiles) / DVE stt (5/16)
    loss_i = A / s^2            (host, while summing partials)
Host: shard batch over 8 cores, combine a/s^2, divide by B*C.

Raw bass (no TileContext): the container's walrus rejects Tile's epilogue
(EVENT_SEMAPHORE_RANGE_CLEAR encoding + many-wait Drain), so semaphores are
managed manually.
"""

import numpy as np

B, C = 16384, 1000
N_CORES = 8
P = 128                    # SBUF partitions
RT = (B // N_CORES) // P   # row-tiles per core = 16
CHUNK = 2                  # row-tiles per input DMA (1 MiB chunks)
NB_E = 4                   # e ring slots
NB_ST = 4                  # state ring slots
SQ_DVE = frozenset({1, 4, 7, 10, 13})   # tiles squared on VectorE; rest ScalarE

_cache = {}


def _build():
    import concourse.bass as bass
    import concourse.mybir as mybir

    f32 = mybir.dt.float32
    f16 = mybir.dt.float16
    Alu = mybir.AluOpType
    Act = mybir.ActivationFunctionType

    nc = bass.Bass("TRN2", target_bir_lowering=False, debug=False,
                   num_devices=N_CORES)

    x_h = nc.dram_tensor("x", [RT * P, C], f32, kind="ExternalInput")
    t_h = nc.dram_tensor("t", [P, RT], f32, kind="ExternalInput")
    iota_h = nc.dram_tensor("iota", [P, C], f16, kind="ExternalInput")
    s_h = nc.dram_tensor("s_out", [P, RT], f32, kind="ExternalOutput")
    a_h = nc.dram_tensor("a_out", [P, RT], f32, kind="ExternalOutput")

    # [RT*P, C] viewed as [P, RT, C]: row (t*P + p) -> partition p, slot t
    x_r = x_h.ap().rearrange("(t p) c -> p t c", p=P)

    x_b = nc.alloc_sbuf_tensor("x_b", [P, RT, C], f32)
    e_b = nc.alloc_sbuf_tensor("e_b", [P, NB_E, C], f16)
    imp_b = nc.alloc_sbuf_tensor("imp_b", [P, C], f16)
    st_b = nc.alloc_sbuf_tensor("st_b", [P, NB_ST, C], f16)
    junkv_b = nc.alloc_sbuf_tensor("junkv_b", [P, C], f16)
    junka_b = nc.alloc_sbuf_tensor("junka_b", [P, C], f16)
    iota_b = nc.alloc_sbuf_tensor("iota_b", [P, C], f16)
    t_b = nc.alloc_sbuf_tensor("t_b", [P, RT], f32)
    s_b = nc.alloc_sbuf_tensor("s_b", [P, RT], f32)
    a_b = nc.alloc_sbuf_tensor("a_b", [P, RT], f32)

    dma_c = nc.alloc_semaphore("dma_c")      # const + output DMAs
    dma_x = nc.alloc_semaphore("dma_x")      # x chunks: +16 each
    s_act = nc.alloc_semaphore("s_act")      # exp done: +1 per tile
    s_scan = nc.alloc_semaphore("s_scan")    # scan done: +1 per tile
    s_sq = nc.alloc_semaphore("s_sq")        # ACT square done: +1 each

    n_act_sq = sum(1 for j in range(RT) if j not in SQ_DVE)

    def n_act_sq_upto(i):
        return sum(1 for j in range(i + 1) if j not in SQ_DVE)

    with nc.Block() as block:

        @block.sync
        def _(sync):
            sync.dma_start(out=iota_b.ap(), in_=iota_h.ap()).then_inc(dma_c, 16)
            sync.dma_start(out=t_b.ap(), in_=t_h.ap()).then_inc(dma_c, 16)
            for ch in range(RT // CHUNK):
                sync.dma_start(
                    out=x_b.ap()[:, ch * CHUNK:(ch + 1) * CHUNK, :],
                    in_=x_r[:, ch * CHUNK:(ch + 1) * CHUNK, :],
                ).then_inc(dma_x, 16)
            sync.wait_ge(s_scan, RT)
            sync.wait_ge(s_sq, n_act_sq)
            sync.dma_start(out=s_h.ap(), in_=s_b.ap()).then_inc(dma_c, 16)
            sync.dma_start(out=a_h.ap(), in_=a_b.ap()).then_inc(dma_c, 16)

        @block.scalar
        def _(scalar):
            for i in range(RT):
                if i % CHUNK == 0:
                    scalar.wait_ge(dma_x, 16 * (i // CHUNK + 1))
                if i >= NB_E:
                    scalar.wait_ge(s_scan, i - NB_E + 1)
                nc.scalar.activation(
                    out=e_b.ap()[:, i % NB_E, :], in_=x_b.ap()[:, i, :],
                    func=Act.Exp, accum_out=s_b.ap()[:, i:i + 1],
                ).then_inc(s_act, 1)
                # lagged square of tile i-1 when it belongs to ACT
                j = i - 1
                if j >= 0 and j not in SQ_DVE:
                    scalar.wait_ge(s_scan, j + 1)
                    nc.scalar.activation(
                        out=junka_b.ap(), in_=st_b.ap()[:, j % NB_ST, :],
                        func=Act.Square, accum_out=a_b.ap()[:, j:j + 1],
                    ).then_inc(s_sq, 1)
            j = RT - 1
            if j not in SQ_DVE:
                scalar.wait_ge(s_scan, j + 1)
                nc.scalar.activation(
                    out=junka_b.ap(), in_=st_b.ap()[:, j % NB_ST, :],
                    func=Act.Square, accum_out=a_b.ap()[:, j:j + 1],
                ).then_inc(s_sq, 1)

        @block.vector
        def _(vector):
            for i in range(RT):
                if i == 0:
                    vector.wait_ge(dma_c, 32)
                vector.wait_ge(s_act, i + 1)
                nc.vector.tensor_scalar(
                    out=imp_b.ap(), in0=iota_b.ap(),
                    scalar1=t_b.ap()[:, i:i + 1], scalar2=s_b.ap()[:, i:i + 1],
                    op0=Alu.is_equal, op1=Alu.mult)
                # state slot WAR: ACT square of tile i-NB_ST must be done
                jw = i - NB_ST
                if jw >= 0 and jw not in SQ_DVE:
                    vector.wait_ge(s_sq, n_act_sq_upto(jw))
                nc.vector.tensor_tensor_scan(
                    out=st_b.ap()[:, i % NB_ST, :], data0=e_b.ap()[:, i % NB_E, :],
                    data1=imp_b.ap(), initial=0.0,
                    op0=Alu.add, op1=Alu.subtract).then_inc(s_scan, 1)
                if i in SQ_DVE:
                    nc.vector.scalar_tensor_tensor(
                        out=junkv_b.ap(), in0=st_b.ap()[:, i % NB_ST, :],
                        scalar=1.0, in1=st_b.ap()[:, i % NB_ST, :],
                        op0=Alu.mult, op1=Alu.mult,
                        accum_out=a_b.ap()[:, i:i + 1])

    return nc


def _get_nc():
    if "nc" not in _cache:
        _cache["nc"] = _build()
    return _cache["nc"]


def _make_in_maps(predicted_logits, true_labels):
    x = np.ascontiguousarray(np.asarray(predicted_logits, dtype=np.float32))
    t = np.asarray(true_labels).astype(np.float32)
    assert x.shape == (B, C), x.shape
    assert t.shape == (B,), t.shape
    iota = np.ascontiguousarray(
        np.broadcast_to(np.arange(C, dtype=np.float16), (P, C)))
    rows_per_core = B // N_CORES
    in_maps = []
    for c in range(N_CORES):
        xc = x[c * rows_per_core:(c + 1) * rows_per_core]
        tc_ = t[c * rows_per_core:(c + 1) * rows_per_core]
        t_tile = np.ascontiguousarray(tc_.reshape(RT, P).T)
        in_maps.append({"x": xc, "t": t_tile, "iota": iota})
    return in_maps


def _run(predicted_logits, true_labels, **run_kwargs):
    from concourse.bass_utils import run_bass_kernel_spmd
    nc = _get_nc()
    in_maps = _make_in_maps(predicted_logits, true_labels)
    out = run_bass_kernel_spmd(nc, in_maps, core_ids=list(range(N_CORES)),
                               **run_kwargs)
    total = 0.0
    for r in out.results:
        s = r["s_out"].astype(np.float64)
        a = r["a_out"].astype(np.float64)
        total += (a / (s * s)).sum()
    loss = np.float32(total / (B * C))
    return loss, out


def kernel(predicted_logits, true_labels):
    loss, _ = _run(predicted_logits, true_labels)
    return loss


# revision 10
# speedup vs baseline: 1.1135x; 1.1135x over previous
"""CRPS loss kernel for Trainium2 (8 NeuronCores, pure data parallel).

Math: for each row i with logits x and label t,
    probs = softmax(x);  F = cumsum(probs);  mask_j = 1[j >= t]
    loss_i = sum_j (F_j - mask_j)^2;  output = sum_i loss_i / (B*C)

Device formulation (per row):
    e = exp(x)                  ACT, fused accum_out -> s = sum(e)
    imp_j = s * (iota_j == t)   one tensor_scalar op (DVE)
    st_j = (e_j + st_{j-1}) - imp_j    = cumsum(e)_j - s*mask_j   (DVE scan)
    A = sum_j st_j^2            ACT Square with accum_out
    loss_i = A / s^2            (host, while summing partials)

Engine balance per [128, 1000] tile: DVE = impulse ts (~0.44us) + scan
(~2.15us, 2 cyc/elem); ACT = exp (~1.2us) + square (~1.3us). Both ~41us/core;
DMA-in 8 MB ~22us overlaps.

Raw bass (no TileContext): the container's walrus rejects Tile's epilogue
(EVENT_SEMAPHORE_RANGE_CLEAR encoding + many-wait Drain), so semaphores are
managed manually.
"""

import numpy as np

B, C = 16384, 1000
N_CORES = 8
P = 128                    # SBUF partitions
RT = (B // N_CORES) // P   # row-tiles per core = 16
NB_E = 4                   # e ring slots
NB_ST = 4                  # state ring slots

_cache = {}


def _build():
    import concourse.bass as bass
    import concourse.mybir as mybir

    f32 = mybir.dt.float32
    f16 = mybir.dt.float16
    Alu = mybir.AluOpType
    Act = mybir.ActivationFunctionType

    nc = bass.Bass("TRN2", target_bir_lowering=False, debug=False,
                   num_devices=N_CORES)

    x_h = nc.dram_tensor("x", [RT * P, C], f32, kind="ExternalInput")
    t_h = nc.dram_tensor("t", [P, RT], f32, kind="ExternalInput")
    iota_h = nc.dram_tensor("iota", [P, C], f16, kind="ExternalInput")
    out_h = nc.dram_tensor("out", [P, 2, RT], f32, kind="ExternalOutput")

    # [RT*P, C] viewed as [P, RT, C]: row (t*P + p) -> partition p, slot t
    x_r = x_h.ap().rearrange("(t p) c -> p t c", p=P)

    x_b = nc.alloc_sbuf_tensor("x_b", [P, RT, C], f32)
    e_b = nc.alloc_sbuf_tensor("e_b", [P, NB_E, C], f16)
    imp_b = nc.alloc_sbuf_tensor("imp_b", [P, C], f16)
    st_b = nc.alloc_sbuf_tensor("st_b", [P, NB_ST, C], f16)
    junka_b = nc.alloc_sbuf_tensor("junka_b", [P, C], f16)
    iota_b = nc.alloc_sbuf_tensor("iota_b", [P, C], f16)
    t_b = nc.alloc_sbuf_tensor("t_b", [P, RT], f32)
    out_b = nc.alloc_sbuf_tensor("out_b", [P, 2, RT], f32)
    s_ap = lambda i: out_b.ap()[:, 0, i:i + 1]
    a_ap = lambda i: out_b.ap()[:, 1, i:i + 1]

    dma_c = nc.alloc_semaphore("dma_c")      # const + output DMAs
    dma_x = nc.alloc_semaphore("dma_x")      # x tiles: +16 each
    s_act = nc.alloc_semaphore("s_act")      # exp done: +1 per tile
    s_scan = nc.alloc_semaphore("s_scan")    # scan done: +1 per tile
    s_sq = nc.alloc_semaphore("s_sq")        # ACT square done: +1 each

    with nc.Block() as block:

        @block.sync
        def _(sync):
            sync.dma_start(out=iota_b.ap(), in_=iota_h.ap()).then_inc(dma_c, 16)
            sync.dma_start(out=t_b.ap(), in_=t_h.ap()).then_inc(dma_c, 16)
            for i in range(RT):
                sync.dma_start(out=x_b.ap()[:, i, :],
                               in_=x_r[:, i, :]).then_inc(dma_x, 16)
            sync.wait_ge(s_sq, RT)
            sync.dma_start(out=out_h.ap(), in_=out_b.ap()).then_inc(dma_c, 16)

        @block.scalar
        def _(scalar):
            for i in range(RT):
                scalar.wait_ge(dma_x, 16 * (i + 1))
                if i >= NB_E:
                    scalar.wait_ge(s_scan, i - NB_E + 1)
                nc.scalar.activation(
                    out=e_b.ap()[:, i % NB_E, :], in_=x_b.ap()[:, i, :],
                    func=Act.Exp, accum_out=s_ap(i),
                ).then_inc(s_act, 1)
                # lagged square of tile i-1
                j = i - 1
                if j >= 0:
                    scalar.wait_ge(s_scan, j + 1)
                    nc.scalar.activation(
                        out=junka_b.ap(), in_=st_b.ap()[:, j % NB_ST, :],
                        func=Act.Square, accum_out=a_ap(j),
                    ).then_inc(s_sq, 1)
            j = RT - 1
            scalar.wait_ge(s_scan, j + 1)
            nc.scalar.activation(
                out=junka_b.ap(), in_=st_b.ap()[:, j % NB_ST, :],
                func=Act.Square, accum_out=a_ap(j),
            ).then_inc(s_sq, 1)

        @block.vector
        def _(vector):
            for i in range(RT):
                if i == 0:
                    vector.wait_ge(dma_c, 32)
                vector.wait_ge(s_act, i + 1)
                nc.vector.tensor_scalar(
                    out=imp_b.ap(), in0=iota_b.ap(),
                    scalar1=t_b.ap()[:, i:i + 1], scalar2=s_ap(i),
                    op0=Alu.is_equal, op1=Alu.mult)
                # state slot WAR: ACT square of tile i-NB_ST must be done
                if i >= NB_ST:
                    vector.wait_ge(s_sq, i - NB_ST + 1)
                nc.vector.tensor_tensor_scan(
                    out=st_b.ap()[:, i % NB_ST, :],
                    data0=e_b.ap()[:, i % NB_E, :],
                    data1=imp_b.ap(), initial=0.0,
                    op0=Alu.add, op1=Alu.subtract).then_inc(s_scan, 1)

    return nc


def _get_nc():
    if "nc" not in _cache:
        _cache["nc"] = _build()
    return _cache["nc"]


def _make_in_maps(predicted_logits, true_labels):
    x = np.ascontiguousarray(np.asarray(predicted_logits, dtype=np.float32))
    t = np.asarray(true_labels).astype(np.float32)
    assert x.shape == (B, C), x.shape
    assert t.shape == (B,), t.shape
    iota = np.ascontiguousarray(
        np.broadcast_to(np.arange(C, dtype=np.float16), (P, C)))
    rows_per_core = B // N_CORES
    in_maps = []
    for c in range(N_CORES):
        xc = x[c * rows_per_core:(c + 1) * rows_per_core]
        tc_ = t[c * rows_per_core:(c + 1) * rows_per_core]
        t_tile = np.ascontiguousarray(tc_.reshape(RT, P).T)
        in_maps.append({"x": xc, "t": t_tile, "iota": iota})
    return in_maps


def _run(predicted_logits, true_labels, **run_kwargs):
    from concourse.bass_utils import run_bass_kernel_spmd
    nc = _get_nc()
    in_maps = _make_in_maps(predicted_logits, true_labels)
    out = run_bass_kernel_spmd(nc, in_maps, core_ids=list(range(N_CORES)),
                               **run_kwargs)
    total = 0.0
    for r in out.results:
        s = r["out"][:, 0, :].astype(np.float64)
        a = r["out"][:, 1, :].astype(np.float64)
        total += (a / (s * s)).sum()
    loss = np.float32(total / (B * C))
    return loss, out


def kernel(predicted_logits, true_labels):
    loss, _ = _run(predicted_logits, true_labels)
    return loss


# revision 22
# speedup vs baseline: 1.1649x; 1.0462x over previous
"""CRPS loss kernel for Trainium2 (8 NeuronCores, pure data parallel).

Math: for each row i with logits x and label t,
    probs = softmax(x);  F = cumsum(probs);  mask_j = 1[j >= t]
    loss_i = sum_j (F_j - mask_j)^2;  output = sum_i loss_i / (B*C)

Device formulation (per row):
    e = exp(x)                  ACT, fused accum_out -> s = sum(e)
    imp_j = s * (iota_j == t)   one tensor_scalar op (DVE)
    st_j = (e_j + st_{j-1}) - imp_j    = cumsum(e)_j - s*mask_j   (DVE scan)
    A = sum_j st_j^2            ACT Square with accum_out
    loss_i = A / s^2            (host, while summing partials)

Engine balance per [128, 1000] tile: DVE = impulse ts (~0.44us) + scan
(~2.15us, 2 cyc/elem); ACT = exp (~1.2us) + square (~1.3us). Both ~41us/core;
DMA-in 8 MB ~22us overlaps.

Raw bass (no TileContext): the container's walrus rejects Tile's epilogue
(EVENT_SEMAPHORE_RANGE_CLEAR encoding + many-wait Drain), so semaphores are
managed manually.
"""

import numpy as np

B, C = 16384, 1000
N_CORES = 8
P = 128                    # SBUF partitions
RT = (B // N_CORES) // P   # row-tiles per core = 16
NB_E = 6                   # e ring slots
NB_ST = 6                  # state ring slots

_cache = {}


def _build():
    import concourse.bass as bass
    import concourse.mybir as mybir

    f32 = mybir.dt.float32
    f16 = mybir.dt.float16
    Alu = mybir.AluOpType
    Act = mybir.ActivationFunctionType

    nc = bass.Bass("TRN2", target_bir_lowering=False, debug=False,
                   num_devices=N_CORES)

    x_h = nc.dram_tensor("x", [RT * P, C], f32, kind="ExternalInput")
    t_h = nc.dram_tensor("t", [P, RT], f32, kind="ExternalInput")
    iota_h = nc.dram_tensor("iota", [P, C], f16, kind="ExternalInput")
    out_h = nc.dram_tensor("out", [P, 2, RT], f32, kind="ExternalOutput")

    # [RT*P, C] viewed as [P, RT, C]: row (t*P + p) -> partition p, slot t
    x_r = x_h.ap().rearrange("(t p) c -> p t c", p=P)

    x_b = nc.alloc_sbuf_tensor("x_b", [P, RT, C], f32)
    e_b = nc.alloc_sbuf_tensor("e_b", [P, NB_E, C], f16)
    imp_b = nc.alloc_sbuf_tensor("imp_b", [P, C], f16)
    st_b = nc.alloc_sbuf_tensor("st_b", [P, NB_ST, C], f16)
    junka_b = nc.alloc_sbuf_tensor("junka_b", [P, C], f16)
    junkv_b = nc.alloc_sbuf_tensor("junkv_b", [P, C], f16)
    iota_b = nc.alloc_sbuf_tensor("iota_b", [P, C], f16)
    t_b = nc.alloc_sbuf_tensor("t_b", [P, RT], f32)
    out_b = nc.alloc_sbuf_tensor("out_b", [P, 2, RT], f32)
    s0h_b = nc.alloc_sbuf_tensor("s0h_b", [P, 2], f32)
    s_ap = lambda i: out_b.ap()[:, 0, i:i + 1]
    a_ap = lambda i: out_b.ap()[:, 1, i:i + 1]

    dma_out = nc.alloc_semaphore("dma_out")  # output DMA
    dma_iota = nc.alloc_semaphore("dma_iota")
    dma_t = nc.alloc_semaphore("dma_t")
    # one semaphore per x-DMA instruction: per-engine completion increments
    # mix across queued DMAs on one semaphore, so a shared counter can hit
    # 16*(n+1) before chunk n has fully landed
    dma_xs = [nc.alloc_semaphore(f"dma_x{k}") for k in range(8)]
    s_act = nc.alloc_semaphore("s_act")      # every ACT compute op: +1
    s_scan = nc.alloc_semaphore("s_scan")    # every DVE op in main loop: +1

    # progressive x-DMA chunks: tile 0 in two halves (fast pipeline start),
    # then growing chunks (amortize the ~600ns per-DMA sequencer issue cost)
    chunks = [1, 1, 2, 4, 4, 4]
    assert sum(chunks) == RT
    starts = [sum(chunks[:k]) for k in range(len(chunks))]
    chunk_of = [max(k for k in range(len(chunks)) if starts[k] <= i)
                for i in range(RT)]
    # x-DMA instruction index whose completion covers tile i (half DMAs are
    # instructions 0 and 1; chunk k>=1 is instruction k+1)
    x_instr = [chunk_of[i] + 1 for i in range(RT)]

    H = C // 2

    # ---- ACT stream with position bookkeeping -------------------------
    # Every compute op increments s_act.  A consumer of an op's accum_out
    # waits for the NEXT op's increment: the accumulator drain of op k is
    # ordered before op k+1 completes on the in-order engine, so this
    # fences the SBUF write of the accumulator regardless of where the
    # lowered sem update sits inside op k itself.
    act_ops = []          # list of ("exp", i) / ("sq", j) / ("aux", ...)
    pos_exp = {}
    pos_sq = {}

    def act_emit(kind, idx, fn):
        fn()
        act_ops.append((kind, idx))
        pos = len(act_ops)
        if kind == "exp":
            pos_exp[idx] = pos
        elif kind == "sq":
            pos_sq[idx] = pos
        return pos

    LEAD = 3

    def emit_exp(i):
        if i == 0:
            def f0a():
                nc.scalar.wait_ge(dma_xs[0], 16)
                nc.scalar.activation(
                    out=e_b.ap()[:, 0, 0:H], in_=x_b.ap()[:, 0, 0:H],
                    func=Act.Exp, accum_out=s0h_b.ap()[:, 0:1],
                ).then_inc(s_act, 1)
            act_emit("aux", None, f0a)

            def f0b():
                nc.scalar.wait_ge(dma_xs[1], 16)
                nc.scalar.activation(
                    out=e_b.ap()[:, 0, H:C], in_=x_b.ap()[:, 0, H:C],
                    func=Act.Exp, accum_out=s0h_b.ap()[:, 1:2],
                ).then_inc(s_act, 1)
            act_emit("exp", 0, f0b)

            def f0c():
                # tiny successor op: fences exp_0b's accumulator early so
                # the DVE does not have to wait for exp_1
                nc.scalar.activation(
                    out=junka_b.ap()[:, 0:8], in_=junka_b.ap()[:, 0:8],
                    func=Act.Exp).then_inc(s_act, 1)
            act_emit("aux", None, f0c)
            return

        def f():
            if chunk_of[i] != chunk_of[i - 1]:
                nc.scalar.wait_ge(dma_xs[x_instr[i]], 16)
            if i >= NB_E:
                # WAR on the e ring slot: scan of tile i-NB_E must be done
                nc.scalar.wait_ge(s_scan, dve_pos_scan[i - NB_E])
            nc.scalar.activation(
                out=e_b.ap()[:, i % NB_E, :], in_=x_b.ap()[:, i, :],
                func=Act.Exp, accum_out=s_ap(i),
            ).then_inc(s_act, 1)
        act_emit("exp", i, f)

    def emit_sq(j):
        def f():
            nc.scalar.wait_ge(s_scan, dve_pos_scan[j])
            nc.scalar.activation(
                out=junka_b.ap(), in_=st_b.ap()[:, j % NB_ST, :],
                func=Act.Square, accum_out=a_ap(j),
            ).then_inc(s_act, 1)
        act_emit("sq", j, f)

    # ---- DVE stream positions (computed up front; every op incs s_scan)
    # per tile i >= 1: [is_eq+mult, scan]; tile 0: [s-add, is_eq, scan];
    # tail: [stt square of tile 15, fence memset]
    dve_pos_scan = {}
    p = 0
    p += 3                      # tile 0: add, imp, scan
    dve_pos_scan[0] = p
    for i in range(1, RT):
        p += 2
        dve_pos_scan[i] = p
    pos_stt = p + 1
    pos_dve_fence = p + 2

    # ---- emit ACT stream ----------------------------------------------
    # dummy first: pre-trigger the exp/square table load during DMA wait
    nc.scalar.activation(out=junka_b.ap()[:, 0:8], in_=junka_b.ap()[:, 0:8],
                         func=Act.Exp)
    # constants via the ACT HWDGE queue (sync queue starts on x tile 0)
    nc.scalar.dma_start(out=iota_b.ap(), in_=iota_h.ap()).then_inc(dma_iota, 16)
    nc.scalar.dma_start(out=t_b.ap(), in_=t_h.ap()).then_inc(dma_t, 16)
    for i in range(RT):
        emit_exp(i)
        if i >= LEAD:
            emit_sq(i - LEAD)
    for j in range(RT - LEAD, RT - 1):
        emit_sq(j)
    # trailing fence op: its inc certifies sq_{RT-2}'s accumulator write
    def f_fence():
        nc.scalar.activation(out=junka_b.ap()[:, 0:8],
                             in_=junka_b.ap()[:, 0:8],
                             func=Act.Exp).then_inc(s_act, 1)
    act_emit("aux", None, f_fence)
    n_act = len(act_ops)

    # ---- Sync (SP) stream: x DMAs + final output ----------------------
    nc.sync.dma_start(out=x_b.ap()[:, 0, 0:H],
                      in_=x_r[:, 0, 0:H]).then_inc(dma_xs[0], 16)
    nc.sync.dma_start(out=x_b.ap()[:, 0, H:C],
                      in_=x_r[:, 0, H:C]).then_inc(dma_xs[1], 16)
    for k, ch in enumerate(chunks):
        if k == 0:
            continue
        nc.sync.dma_start(
            out=x_b.ap()[:, starts[k]:starts[k] + ch, :],
            in_=x_r[:, starts[k]:starts[k] + ch, :],
        ).then_inc(dma_xs[k + 1], 16)
    nc.sync.wait_ge(s_act, n_act)            # fences all ACT accums
    nc.sync.wait_ge(s_scan, pos_dve_fence)   # fences the DVE stt accum
    nc.sync.dma_start(out=out_h.ap(), in_=out_b.ap()).then_inc(dma_out, 16)

    # ---- DVE stream ----------------------------------------------------
    for i in range(RT):
        if i == 0:
            nc.vector.wait_ge(dma_iota, 16)
            nc.vector.wait_ge(dma_t, 16)
            # wait for the op AFTER exp_0 (fences exp_0b's accumulator)
            nc.vector.wait_ge(s_act, pos_exp[0] + 1)
            nc.vector.tensor_tensor(s_ap(0), s0h_b.ap()[:, 0:1],
                                    s0h_b.ap()[:, 1:2],
                                    Alu.add).then_inc(s_scan, 1)
        else:
            nc.vector.wait_ge(s_act, pos_exp[i] + 1)
        nc.vector.tensor_scalar(
            out=imp_b.ap(), in0=iota_b.ap(),
            scalar1=t_b.ap()[:, i:i + 1], scalar2=s_ap(i),
            op0=Alu.is_equal, op1=Alu.mult).then_inc(s_scan, 1)
        # st ring WAR: the square of tile i-NB_ST must have read its slot
        if i >= NB_ST:
            nc.vector.wait_ge(s_act, pos_sq[i - NB_ST])
        nc.vector.tensor_tensor_scan(
            out=st_b.ap()[:, i % NB_ST, :],
            data0=e_b.ap()[:, i % NB_E, :],
            data1=imp_b.ap(), initial=0.0,
            op0=Alu.add, op1=Alu.subtract).then_inc(s_scan, 1)
    j = RT - 1
    nc.vector.scalar_tensor_tensor(
        out=junkv_b.ap(), in0=st_b.ap()[:, j % NB_ST, :], scalar=1.0,
        in1=st_b.ap()[:, j % NB_ST, :], op0=Alu.mult, op1=Alu.mult,
        accum_out=a_ap(j)).then_inc(s_scan, 1)
    # trailing DVE fence: certifies the stt accumulator write
    nc.vector.memset(junkv_b.ap()[:, 0:8], 0.0)
    nc.vector.tensor_scalar(out=junkv_b.ap()[:, 0:8],
                            in0=junkv_b.ap()[:, 0:8], scalar1=1.0,
                            scalar2=None,
                            op0=Alu.mult).then_inc(s_scan, 1)

    assert dve_pos_scan[0] == 3
    return nc


def _get_nc():
    if "nc" not in _cache:
        _cache["nc"] = _build()
    return _cache["nc"]


def _make_in_maps(predicted_logits, true_labels):
    x = np.ascontiguousarray(np.asarray(predicted_logits, dtype=np.float32))
    t = np.asarray(true_labels).astype(np.float32)
    assert x.shape == (B, C), x.shape
    assert t.shape == (B,), t.shape
    iota = np.ascontiguousarray(
        np.broadcast_to(np.arange(C, dtype=np.float16), (P, C)))
    rows_per_core = B // N_CORES
    in_maps = []
    for c in range(N_CORES):
        xc = x[c * rows_per_core:(c + 1) * rows_per_core]
        tc_ = t[c * rows_per_core:(c + 1) * rows_per_core]
        t_tile = np.ascontiguousarray(tc_.reshape(RT, P).T)
        in_maps.append({"x": xc, "t": t_tile, "iota": iota})
    return in_maps


def _run(predicted_logits, true_labels, **run_kwargs):
    from concourse.bass_utils import run_bass_kernel_spmd
    nc = _get_nc()
    in_maps = _make_in_maps(predicted_logits, true_labels)
    out = run_bass_kernel_spmd(nc, in_maps, core_ids=list(range(N_CORES)),
                               **run_kwargs)
    total = 0.0
    for r in out.results:
        s = r["out"][:, 0, :].astype(np.float64)
        a = r["out"][:, 1, :].astype(np.float64)
        total += (a / (s * s)).sum()
    loss = np.float32(total / (B * C))
    return loss, out


def kernel(predicted_logits, true_labels):
    loss, _ = _run(predicted_logits, true_labels)
    return loss
